# revision 10
# baseline (speedup 1.0000x reference)
"""CosGaussianKernelDiagonal on 8 Trainium2 NeuronCores.

out[b,n,m] = cos(mx[b,n] - my[b,m]) * exp(-0.5 * max(x2[b,n] + y2[b,m] - 2*xy[b,n,m], 0))

with mx = x@mu, my = y@mu, x_ = x*exp(0.5*logs2diag), x2 = |x_|^2, xy = x_ . y_.

Device-side restructuring (per [128,512] output tile):
  out = (cx[n]*cy'[m] + sx[n]*sy'[m]) * exp(xy[n,m] - 0.5*x2[n])
with cx = cos(mx), sx = sin(mx), cy' = cos(my)*exp(-0.5*y2), sy' = sin(my)*exp(-0.5*y2)
precomputed on host in float64 (O(N+M) work). Device does, per tile:
  - xy via one K=128 fp16 matmul [xh;xl] x [yh;yh]  (x split hi/lo in fp16, so x is
    carried to ~22 mantissa bits; only x.(y - fp16(y)) is dropped, ~3e-3 absolute)
  - rank-2 cos term via one K=6 bf16 matmul (hi/lo split of the four vectors;
    bf16 keeps fp32 exponent range -- cy' underflows fp16)
  - one ACT pass exp(psum + bias_n), bias_n = -0.5*x2 exact fp32 per-partition
  - one DVE pass multiplying the two
Sharding: 8 cores = (batch b, n-half). Each core computes a [2048, 4096] block.
"""

import sys

if "/opt/trn_rl_repo" not in sys.path:
    sys.path.insert(0, "/opt/trn_rl_repo")

import numpy as np

B, N, M, D = 4, 4096, 4096, 64
NSH = N // 2          # n rows per core
NB = NSH // 128       # 16 n-blocks per core
MC = 4                # psum chunks per n-block
MCW = M // MC         # 1024 columns per chunk
MT = MCW // 512       # 512-wide matmuls per chunk

SCHEME = "precise"    # "fast": 2 matmuls/tile (fp16 split); "precise": 3 (bf16 split)

_CACHE = {}


def _build(loop_n=1, scheme=None):
    scheme = scheme or SCHEME
    key = ("nc", loop_n, scheme)
    if key in _CACHE:
        return _CACHE[key]

    import concourse.bacc as bacc
    import concourse.tile as tile
    from concourse import mybir

    f32 = mybir.dt.float32
    bf16 = mybir.dt.bfloat16
    fp16 = mybir.dt.float16
    xdt = fp16 if scheme == "fast" else bf16

    nc = bacc.Bacc("TRN2", target_bir_lowering=False, debug=False, num_devices=8)

    d_xhl = nc.dram_tensor("xhl", [128, NSH], xdt, kind="ExternalInput")
    d_yhh = nc.dram_tensor("yhh", [128, M], xdt, kind="ExternalInput")
    if scheme == "precise":
        d_yl = nc.dram_tensor("yl", [D, M], bf16, kind="ExternalInput")
    d_csx = nc.dram_tensor("csx", [6, NSH], bf16, kind="ExternalInput")
    d_csy = nc.dram_tensor("csy", [6, M], bf16, kind="ExternalInput")
    d_bias = nc.dram_tensor("bias", [128, NB], f32, kind="ExternalInput")
    d_out = nc.dram_tensor("out", [NSH, M], f32, kind="ExternalOutput")

    with tile.TileContext(nc) as tc:
        with tc.tile_pool(name="singles", bufs=1) as singles, \
             tc.tile_pool(name="work", bufs=3) as work, \
             tc.tile_pool(name="outp", bufs=3) as outp, \
             tc.tile_pool(name="psE", bufs=2, space="PSUM") as psE_pool, \
             tc.tile_pool(name="psC", bufs=2, space="PSUM") as psC_pool:

            def body(_iv=None):
                t_xhl = singles.tile([128, NSH], xdt)
                nc.sync.dma_start(out=t_xhl[:], in_=d_xhl[:, :])
                t_csx = singles.tile([6, NSH], bf16)
                nc.sync.dma_start(out=t_csx[:], in_=d_csx[:, :])
                t_csy = singles.tile([6, M], bf16)
                nc.sync.dma_start(out=t_csy[:], in_=d_csy[:, :])
                t_bias = singles.tile([128, NB], f32)
                nc.sync.dma_start(out=t_bias[:], in_=d_bias[:, :])
                # y-side loaded in column chunks so compute can start early
                t_yhh = singles.tile([128, M], xdt)
                if scheme == "precise":
                    t_yl = singles.tile([D, M], bf16)
                for mc in range(MC):
                    m0 = mc * MCW
                    nc.sync.dma_start(out=t_yhh[:, m0:m0 + MCW], in_=d_yhh[:, m0:m0 + MCW])
                    if scheme == "precise":
                        nc.sync.dma_start(out=t_yl[:, m0:m0 + MCW], in_=d_yl[:, m0:m0 + MCW])

                for nb in range(NB):
                    n0 = nb * 128
                    for mc in range(MC):
                        m0 = mc * MCW
                        pE = psE_pool.tile([128, MCW], f32, tag="pE")
                        pC = psC_pool.tile([128, MCW], f32, tag="pC")
                        # group matmuls by stationary operand to amortize LDWEIGHTS
                        if scheme == "fast":
                            for mt in range(MT):
                                s0 = mt * 512
                                nc.tensor.matmul(
                                    pE[:, s0:s0 + 512],
                                    t_xhl[:, n0:n0 + 128],
                                    t_yhh[:, m0 + s0:m0 + s0 + 512],
                                    start=True, stop=True, skip_group_check=True,
                                )
                        else:
                            for mt in range(MT):
                                s0 = mt * 512
                                nc.tensor.matmul(
                                    pE[:, s0:s0 + 512],
                                    t_xhl[:, n0:n0 + 128],
                                    t_yhh[:, m0 + s0:m0 + s0 + 512],
                                    start=True, stop=False,
                                )
                                nc.tensor.matmul(
                                    pE[:, s0:s0 + 512],
                                    t_xhl[:D, n0:n0 + 128],
                                    t_yl[:, m0 + s0:m0 + s0 + 512],
                                    start=False, stop=True,
                                )
                                nc.tensor.matmul(
                                    pC[:, s0:s0 + 512],
                                    t_csx[:, n0:n0 + 128],
                                    t_csy[:, m0 + s0:m0 + s0 + 512],
                                    start=True, stop=True,
                                )
                        if scheme == "fast":
                            for mt in range(MT):
                                s0 = mt * 512
                                nc.tensor.matmul(
                                    pC[:, s0:s0 + 512],
                                    t_csx[:, n0:n0 + 128],
                                    t_csy[:, m0 + s0:m0 + s0 + 512],
                                    start=True, stop=True, skip_group_check=True,
                                )
                        g = work.tile([128, MCW], f32, tag="g")
                        nc.scalar.activation(
                            g[:], pE[:], mybir.ActivationFunctionType.Exp,
                            bias=t_bias[:, nb:nb + 1], scale=1.0,
                        )
                        o = outp.tile([128, MCW], f32, tag="o")
                        nc.vector.tensor_tensor(
                            out=o[:], in0=g[:], in1=pC[:],
                            op=mybir.AluOpType.mult,
                        )
                        nc.sync.dma_start(out=d_out[n0:n0 + 128, m0:m0 + MCW], in_=o[:])

            if loop_n == 1:
                body()
            else:
                with tc.For_i(0, loop_n, 1) as iv:
                    body(iv)

    nc.compile()
    _CACHE[key] = nc
    return nc


def _split(a32, dt):
    hi = a32.astype(dt)
    lo = (a32 - hi.astype(np.float32)).astype(dt)
    return hi, lo


def make_in_maps(x, y, mu, logs2diag, scheme=None):
    import ml_dtypes
    scheme = scheme or SCHEME
    xdt = np.float16 if scheme == "fast" else ml_dtypes.bfloat16
    bf = ml_dtypes.bfloat16

    x64 = np.asarray(x, dtype=np.float64)
    y64 = np.asarray(y, dtype=np.float64)
    mu64 = np.asarray(mu, dtype=np.float64).reshape(D)
    ls64 = np.asarray(logs2diag, dtype=np.float64)

    s = np.exp(0.5 * ls64)                      # [D]
    x_ = x64 * s                                # [B,N,D]
    y_ = y64 * s
    mx = x64 @ mu64                             # [B,N]
    my = y64 @ mu64                             # [B,M]
    x2 = (x_ * x_).sum(-1)                      # [B,N]
    y2 = (y_ * y_).sum(-1)                      # [B,M]
    gy = np.exp(-0.5 * y2)                      # [B,M]

    in_maps = []
    for c in range(8):
        b, nh = c // 2, c % 2
        nsl = slice(nh * NSH, (nh + 1) * NSH)
        xT = np.ascontiguousarray(x_[b, nsl].T, dtype=np.float32)        # [D, NSH]
        yT = np.ascontiguousarray(y_[b].T, dtype=np.float32)             # [D, M]
        xh, xl = _split(xT, xdt)
        yh = yT.astype(xdt)
        xhl = np.concatenate([xh, xl], axis=0)                            # [128, NSH]
        yhh = np.concatenate([yh, yh], axis=0)                            # [128, M]

        cx = np.cos(mx[b, nsl]).astype(np.float32)
        sx = np.sin(mx[b, nsl]).astype(np.float32)
        cy = (np.cos(my[b]) * gy[b]).astype(np.float32)
        sy = (np.sin(my[b]) * gy[b]).astype(np.float32)
        cxh, cxl = _split(cx, bf)
        sxh, sxl = _split(sx, bf)
        cyh, cyl = _split(cy, bf)
        syh, syl = _split(sy, bf)
        csx = np.stack([cxh, sxh, cxh, sxh, cxl, sxl])                    # [6, NSH]
        csy = np.stack([cyh, syh, cyl, syl, cyh, syh])                    # [6, M]

        bias = np.ascontiguousarray(
            (-0.5 * x2[b, nsl]).reshape(NB, 128).T, dtype=np.float32)    # [128, NB]
        im = dict(xhl=xhl, yhh=yhh, csx=csx, csy=csy, bias=bias)
        if scheme == "precise":
            yl = (yT - yh.astype(np.float32)).astype(bf)
            im["yl"] = yl
        in_maps.append(im)
    return in_maps


def kernel(x, y, mu, logs2diag):
    from concourse.bass_utils import run_bass_kernel_spmd

    nc = _build()
    in_maps = make_in_maps(x, y, mu, logs2diag)
    res = run_bass_kernel_spmd(nc, in_maps, core_ids=list(range(8)))

    out = np.empty((B, N, M), dtype=np.float32)
    for c in range(8):
        b, nh = c // 2, c % 2
        out[b, nh * NSH:(nh + 1) * NSH, :] = res.results[c]["out"]
    return out


# revision 11
# speedup vs baseline: 1.5446x; 1.5446x over previous
"""CosGaussianKernelDiagonal on 8 Trainium2 NeuronCores.

out[b,n,m] = cos(mx[b,n] - my[b,m]) * exp(-0.5 * max(x2[b,n] + y2[b,m] - 2*xy[b,n,m], 0))

with mx = x@mu, my = y@mu, x_ = x*exp(0.5*logs2diag), x2 = |x_|^2, xy = x_ . y_.

Device-side restructuring (per [128,512] output tile):
  out = (cx[n]*cy'[m] + sx[n]*sy'[m]) * exp(xy[n,m] - 0.5*x2[n])
with cx = cos(mx), sx = sin(mx), cy' = cos(my)*exp(-0.5*y2), sy' = sin(my)*exp(-0.5*y2)
precomputed on host in float64 (O(N+M) work). Device does, per tile:
  - xy via one K=128 fp16 matmul [xh;xl] x [yh;yh]  (x split hi/lo in fp16, so x is
    carried to ~22 mantissa bits; only x.(y - fp16(y)) is dropped, ~3e-3 absolute)
  - rank-2 cos term via one K=6 bf16 matmul (hi/lo split of the four vectors;
    bf16 keeps fp32 exponent range -- cy' underflows fp16)
  - one ACT pass exp(psum + bias_n), bias_n = -0.5*x2 exact fp32 per-partition
  - one DVE pass multiplying the two
Sharding: 8 cores = (batch b, n-half). Each core computes a [2048, 4096] block.
"""

import sys

if "/opt/trn_rl_repo" not in sys.path:
    sys.path.insert(0, "/opt/trn_rl_repo")

import numpy as np

B, N, M, D = 4, 4096, 4096, 64
NSH = N // 2          # n rows per core
NB = NSH // 128       # 16 n-blocks per core
MC = 4                # psum chunks per n-block
MCW = M // MC         # 1024 columns per chunk
MT = MCW // 512       # 512-wide matmuls per chunk

SCHEME = "fast"       # "fast": 2 matmuls/tile (fp16 split); "precise": 3 (bf16 split)

_CACHE = {}


def _build(loop_n=1, scheme=None):
    scheme = scheme or SCHEME
    key = ("nc", loop_n, scheme)
    if key in _CACHE:
        return _CACHE[key]

    import concourse.bacc as bacc
    import concourse.tile as tile
    from concourse import mybir

    f32 = mybir.dt.float32
    bf16 = mybir.dt.bfloat16
    fp16 = mybir.dt.float16
    xdt = fp16 if scheme == "fast" else bf16

    nc = bacc.Bacc("TRN2", target_bir_lowering=False, debug=False, num_devices=8)

    d_xhl = nc.dram_tensor("xhl", [128, NSH], xdt, kind="ExternalInput")
    d_yhh = nc.dram_tensor("yhh", [128, M], xdt, kind="ExternalInput")
    if scheme == "precise":
        d_yl = nc.dram_tensor("yl", [D, M], bf16, kind="ExternalInput")
    d_csx = nc.dram_tensor("csx", [6, NSH], bf16, kind="ExternalInput")
    d_csy = nc.dram_tensor("csy", [6, M], bf16, kind="ExternalInput")
    d_bias = nc.dram_tensor("bias", [128, NB], f32, kind="ExternalInput")
    d_out = nc.dram_tensor("out", [NSH, M], f32, kind="ExternalOutput")

    with tile.TileContext(nc) as tc:
        with tc.tile_pool(name="singles", bufs=1) as singles, \
             tc.tile_pool(name="work", bufs=3) as work, \
             tc.tile_pool(name="outp", bufs=3) as outp, \
             tc.tile_pool(name="psE", bufs=2, space="PSUM") as psE_pool, \
             tc.tile_pool(name="psC", bufs=2, space="PSUM") as psC_pool:

            def body(_iv=None):
                t_xhl = singles.tile([128, NSH], xdt)
                nc.sync.dma_start(out=t_xhl[:], in_=d_xhl[:, :])
                t_csx = singles.tile([6, NSH], bf16)
                nc.sync.dma_start(out=t_csx[:], in_=d_csx[:, :])
                t_csy = singles.tile([6, M], bf16)
                nc.sync.dma_start(out=t_csy[:], in_=d_csy[:, :])
                t_bias = singles.tile([128, NB], f32)
                nc.sync.dma_start(out=t_bias[:], in_=d_bias[:, :])
                # y-side loaded in column chunks so compute can start early
                t_yhh = singles.tile([128, M], xdt)
                if scheme == "precise":
                    t_yl = singles.tile([D, M], bf16)
                for mc in range(MC):
                    m0 = mc * MCW
                    nc.sync.dma_start(out=t_yhh[:, m0:m0 + MCW], in_=d_yhh[:, m0:m0 + MCW])
                    if scheme == "precise":
                        nc.sync.dma_start(out=t_yl[:, m0:m0 + MCW], in_=d_yl[:, m0:m0 + MCW])

                for nb in range(NB):
                    n0 = nb * 128
                    for mc in range(MC):
                        m0 = mc * MCW
                        pE = psE_pool.tile([128, MCW], f32, tag="pE")
                        pC = psC_pool.tile([128, MCW], f32, tag="pC")
                        # group matmuls by stationary operand to amortize LDWEIGHTS
                        if scheme == "fast":
                            for mt in range(MT):
                                s0 = mt * 512
                                nc.tensor.matmul(
                                    pE[:, s0:s0 + 512],
                                    t_xhl[:, n0:n0 + 128],
                                    t_yhh[:, m0 + s0:m0 + s0 + 512],
                                    start=True, stop=True, skip_group_check=True,
                                )
                        else:
                            for mt in range(MT):
                                s0 = mt * 512
                                nc.tensor.matmul(
                                    pE[:, s0:s0 + 512],
                                    t_xhl[:, n0:n0 + 128],
                                    t_yhh[:, m0 + s0:m0 + s0 + 512],
                                    start=True, stop=False,
                                )
                                nc.tensor.matmul(
                                    pE[:, s0:s0 + 512],
                                    t_xhl[:D, n0:n0 + 128],
                                    t_yl[:, m0 + s0:m0 + s0 + 512],
                                    start=False, stop=True,
                                )
                                nc.tensor.matmul(
                                    pC[:, s0:s0 + 512],
                                    t_csx[:, n0:n0 + 128],
                                    t_csy[:, m0 + s0:m0 + s0 + 512],
                                    start=True, stop=True,
                                )
                        if scheme == "fast":
                            for mt in range(MT):
                                s0 = mt * 512
                                nc.tensor.matmul(
                                    pC[:, s0:s0 + 512],
                                    t_csx[:, n0:n0 + 128],
                                    t_csy[:, m0 + s0:m0 + s0 + 512],
                                    start=True, stop=True, skip_group_check=True,
                                )
                        g = work.tile([128, MCW], f32, tag="g")
                        nc.scalar.activation(
                            g[:], pE[:], mybir.ActivationFunctionType.Exp,
                            bias=t_bias[:, nb:nb + 1], scale=1.0,
                        )
                        o = outp.tile([128, MCW], f32, tag="o")
                        nc.vector.tensor_tensor(
                            out=o[:], in0=g[:], in1=pC[:],
                            op=mybir.AluOpType.mult,
                        )
                        nc.sync.dma_start(out=d_out[n0:n0 + 128, m0:m0 + MCW], in_=o[:])

            if loop_n == 1:
                body()
            else:
                with tc.For_i(0, loop_n, 1) as iv:
                    body(iv)

    nc.compile()
    _CACHE[key] = nc
    return nc


def _split(a32, dt):
    hi = a32.astype(dt)
    lo = (a32 - hi.astype(np.float32)).astype(dt)
    return hi, lo


def make_in_maps(x, y, mu, logs2diag, scheme=None):
    import ml_dtypes
    scheme = scheme or SCHEME
    xdt = np.float16 if scheme == "fast" else ml_dtypes.bfloat16
    bf = ml_dtypes.bfloat16

    x64 = np.asarray(x, dtype=np.float64)
    y64 = np.asarray(y, dtype=np.float64)
    mu64 = np.asarray(mu, dtype=np.float64).reshape(D)
    ls64 = np.asarray(logs2diag, dtype=np.float64)

    s = np.exp(0.5 * ls64)                      # [D]
    x_ = x64 * s                                # [B,N,D]
    y_ = y64 * s
    mx = x64 @ mu64                             # [B,N]
    my = y64 @ mu64                             # [B,M]
    x2 = (x_ * x_).sum(-1)                      # [B,N]
    y2 = (y_ * y_).sum(-1)                      # [B,M]
    gy = np.exp(-0.5 * y2)                      # [B,M]

    in_maps = []
    for c in range(8):
        b, nh = c // 2, c % 2
        nsl = slice(nh * NSH, (nh + 1) * NSH)
        xT = np.ascontiguousarray(x_[b, nsl].T, dtype=np.float32)        # [D, NSH]
        yT = np.ascontiguousarray(y_[b].T, dtype=np.float32)             # [D, M]
        xh, xl = _split(xT, xdt)
        yh = yT.astype(xdt)
        xhl = np.concatenate([xh, xl], axis=0)                            # [128, NSH]
        yhh = np.concatenate([yh, yh], axis=0)                            # [128, M]

        cx = np.cos(mx[b, nsl]).astype(np.float32)
        sx = np.sin(mx[b, nsl]).astype(np.float32)
        cy = (np.cos(my[b]) * gy[b]).astype(np.float32)
        sy = (np.sin(my[b]) * gy[b]).astype(np.float32)
        cxh, cxl = _split(cx, bf)
        sxh, sxl = _split(sx, bf)
        cyh, cyl = _split(cy, bf)
        syh, syl = _split(sy, bf)
        csx = np.stack([cxh, sxh, cxh, sxh, cxl, sxl])                    # [6, NSH]
        csy = np.stack([cyh, syh, cyl, syl, cyh, syh])                    # [6, M]

        bias = np.ascontiguousarray(
            (-0.5 * x2[b, nsl]).reshape(NB, 128).T, dtype=np.float32)    # [128, NB]
        im = dict(xhl=xhl, yhh=yhh, csx=csx, csy=csy, bias=bias)
        if scheme == "precise":
            yl = (yT - yh.astype(np.float32)).astype(bf)
            im["yl"] = yl
        in_maps.append(im)
    return in_maps


def kernel(x, y, mu, logs2diag):
    from concourse.bass_utils import run_bass_kernel_spmd

    nc = _build()
    in_maps = make_in_maps(x, y, mu, logs2diag)
    res = run_bass_kernel_spmd(nc, in_maps, core_ids=list(range(8)))

    out = np.empty((B, N, M), dtype=np.float32)
    for c in range(8):
        b, nh = c // 2, c % 2
        out[b, nh * NSH:(nh + 1) * NSH, :] = res.results[c]["out"]
    return out


# revision 14
# speedup vs baseline: 1.7766x; 1.1502x over previous
"""CosGaussianKernelDiagonal on 8 Trainium2 NeuronCores.

out[b,n,m] = cos(mx[b,n] - my[b,m]) * exp(-0.5 * max(x2[b,n] + y2[b,m] - 2*xy[b,n,m], 0))

with mx = x@mu, my = y@mu, x_ = x*exp(0.5*logs2diag), x2 = |x_|^2, xy = x_ . y_.

Device-side restructuring (per [128,512] output tile):
  out = (cx[n]*cy'[m] + sx[n]*sy'[m]) * exp(xy[n,m] - 0.5*x2[n])
with cx = cos(mx), sx = sin(mx), cy' = cos(my)*exp(-0.5*y2), sy' = sin(my)*exp(-0.5*y2)
precomputed on host in float64 (O(N+M) work). Device does, per tile:
  - xy via one K=128 fp16 matmul [xh;xl] x [yh;yh]  (x split hi/lo in fp16, so x is
    carried to ~22 mantissa bits; only x.(y - fp16(y)) is dropped, ~3e-3 absolute)
  - rank-2 cos term via one K=6 bf16 matmul (hi/lo split of the four vectors;
    bf16 keeps fp32 exponent range -- cy' underflows fp16)
  - one ACT pass exp(psum + bias_n), bias_n = -0.5*x2 exact fp32 per-partition
  - one DVE pass multiplying the two
Sharding: 8 cores = (batch b, n-half). Each core computes a [2048, 4096] block.
"""

import sys

if "/opt/trn_rl_repo" not in sys.path:
    sys.path.insert(0, "/opt/trn_rl_repo")

import numpy as np

B, N, M, D = 4, 4096, 4096, 64
NSH = N // 2          # n rows per core
NB = NSH // 128       # 16 n-blocks per core
MC = 4                # psum chunks per n-block
MCW = M // MC         # 1024 columns per chunk
MT = MCW // 512       # 512-wide matmuls per chunk

SCHEME = "fast"       # "fast": 2 matmuls/tile (fp16 split); "precise": 3 (bf16 split)

_CACHE = {}


def _build(loop_n=1, scheme=None, bigdma=False, warmup=0):
    scheme = scheme or SCHEME
    key = ("nc", loop_n, scheme, bigdma, warmup)
    if key in _CACHE:
        return _CACHE[key]

    import concourse.bacc as bacc
    import concourse.tile as tile
    from concourse import mybir

    f32 = mybir.dt.float32
    bf16 = mybir.dt.bfloat16
    fp16 = mybir.dt.float16
    xdt = fp16 if scheme == "fast" else bf16

    nc = bacc.Bacc("TRN2", target_bir_lowering=False, debug=False, num_devices=8)

    d_xhl = nc.dram_tensor("xhl", [128, NSH], xdt, kind="ExternalInput")
    d_yhh = nc.dram_tensor("yhh", [128, M], xdt, kind="ExternalInput")
    if scheme == "precise":
        d_yl = nc.dram_tensor("yl", [D, M], bf16, kind="ExternalInput")
    d_csx = nc.dram_tensor("csx", [6, NSH], bf16, kind="ExternalInput")
    d_csy = nc.dram_tensor("csy", [6, M], bf16, kind="ExternalInput")
    d_bias = nc.dram_tensor("bias", [128, NB], f32, kind="ExternalInput")
    d_out = nc.dram_tensor("out", [NSH, M], f32, kind="ExternalOutput")

    with tile.TileContext(nc) as tc:
        with tc.tile_pool(name="singles", bufs=1) as singles, \
             tc.tile_pool(name="work", bufs=3) as work, \
             tc.tile_pool(name="outp", bufs=3) as outp, \
             tc.tile_pool(name="psE", bufs=2, space="PSUM") as psE_pool, \
             tc.tile_pool(name="psC", bufs=2, space="PSUM") as psC_pool:

            def body(_iv=None):
                if warmup:
                    # keep the PE busy through the input-load phase so HAM is
                    # at 2.4 GHz when the real matmuls start
                    wt = singles.tile([128, 512], xdt, tag="warmtile")
                    nc.vector.memset(wt[:], 0.0)
                    for _ in range(warmup):
                        pw = psE_pool.tile([128, MCW], f32, tag="pE")
                        nc.tensor.matmul(
                            pw[:, 0:512], wt[:, 0:128], wt[:, 0:512],
                            start=True, stop=True, skip_group_check=True,
                        )
                t_xhl = singles.tile([128, NSH], xdt)
                nc.sync.dma_start(out=t_xhl[:], in_=d_xhl[:, :])
                t_csx = singles.tile([6, NSH], bf16)
                nc.sync.dma_start(out=t_csx[:], in_=d_csx[:, :])
                t_csy = singles.tile([6, M], bf16)
                nc.sync.dma_start(out=t_csy[:], in_=d_csy[:, :])
                t_bias = singles.tile([128, NB], f32)
                nc.sync.dma_start(out=t_bias[:], in_=d_bias[:, :])
                # y-side loaded in column chunks so compute can start early
                t_yhh = singles.tile([128, M], xdt)
                if scheme == "precise":
                    t_yl = singles.tile([D, M], bf16)
                for mc in range(MC):
                    m0 = mc * MCW
                    nc.sync.dma_start(out=t_yhh[:, m0:m0 + MCW], in_=d_yhh[:, m0:m0 + MCW])
                    if scheme == "precise":
                        nc.sync.dma_start(out=t_yl[:, m0:m0 + MCW], in_=d_yl[:, m0:m0 + MCW])

                for nb in range(NB):
                    n0 = nb * 128
                    for mc in range(MC):
                        m0 = mc * MCW
                        pE = psE_pool.tile([128, MCW], f32, tag="pE")
                        pC = psC_pool.tile([128, MCW], f32, tag="pC")
                        # group matmuls by stationary operand to amortize LDWEIGHTS
                        if scheme == "fast":
                            for mt in range(MT):
                                s0 = mt * 512
                                nc.tensor.matmul(
                                    pE[:, s0:s0 + 512],
                                    t_xhl[:, n0:n0 + 128],
                                    t_yhh[:, m0 + s0:m0 + s0 + 512],
                                    start=True, stop=True, skip_group_check=True,
                                )
                        else:
                            for mt in range(MT):
                                s0 = mt * 512
                                nc.tensor.matmul(
                                    pE[:, s0:s0 + 512],
                                    t_xhl[:, n0:n0 + 128],
                                    t_yhh[:, m0 + s0:m0 + s0 + 512],
                                    start=True, stop=False,
                                )
                                nc.tensor.matmul(
                                    pE[:, s0:s0 + 512],
                                    t_xhl[:D, n0:n0 + 128],
                                    t_yl[:, m0 + s0:m0 + s0 + 512],
                                    start=False, stop=True,
                                )
                                nc.tensor.matmul(
                                    pC[:, s0:s0 + 512],
                                    t_csx[:, n0:n0 + 128],
                                    t_csy[:, m0 + s0:m0 + s0 + 512],
                                    start=True, stop=True,
                                )
                        if scheme == "fast":
                            for mt in range(MT):
                                s0 = mt * 512
                                nc.tensor.matmul(
                                    pC[:, s0:s0 + 512],
                                    t_csx[:, n0:n0 + 128],
                                    t_csy[:, m0 + s0:m0 + s0 + 512],
                                    start=True, stop=True, skip_group_check=True,
                                )
                        g = work.tile([128, MCW], f32, tag="g")
                        nc.scalar.activation(
                            g[:], pE[:], mybir.ActivationFunctionType.Exp,
                            bias=t_bias[:, nb:nb + 1], scale=1.0,
                        )
                        if bigdma:
                            if mc % 2 == 0:
                                o = outp.tile([128, 2 * MCW], f32, tag="o")
                            h0 = (mc % 2) * MCW
                            nc.vector.tensor_tensor(
                                out=o[:, h0:h0 + MCW], in0=g[:], in1=pC[:],
                                op=mybir.AluOpType.mult,
                            )
                            if mc % 2 == 1:
                                nc.sync.dma_start(
                                    out=d_out[n0:n0 + 128, m0 - MCW:m0 + MCW],
                                    in_=o[:],
                                )
                        else:
                            o = outp.tile([128, MCW], f32, tag="o")
                            nc.vector.tensor_tensor(
                                out=o[:], in0=g[:], in1=pC[:],
                                op=mybir.AluOpType.mult,
                            )
                            nc.sync.dma_start(out=d_out[n0:n0 + 128, m0:m0 + MCW], in_=o[:])

            if loop_n == 1:
                body()
            else:
                with tc.For_i(0, loop_n, 1) as iv:
                    body(iv)

    nc.compile()
    _CACHE[key] = nc
    return nc


def _split(a32, dt):
    hi = a32.astype(dt)
    lo = (a32 - hi.astype(np.float32)).astype(dt)
    return hi, lo


def make_in_maps(x, y, mu, logs2diag, scheme=None):
    import ml_dtypes
    scheme = scheme or SCHEME
    xdt = np.float16 if scheme == "fast" else ml_dtypes.bfloat16
    bf = ml_dtypes.bfloat16

    x64 = np.asarray(x, dtype=np.float64)
    y64 = np.asarray(y, dtype=np.float64)
    mu64 = np.asarray(mu, dtype=np.float64).reshape(D)
    ls64 = np.asarray(logs2diag, dtype=np.float64)

    s = np.exp(0.5 * ls64)                      # [D]
    x_ = x64 * s                                # [B,N,D]
    y_ = y64 * s
    mx = x64 @ mu64                             # [B,N]
    my = y64 @ mu64                             # [B,M]
    x2 = (x_ * x_).sum(-1)                      # [B,N]
    y2 = (y_ * y_).sum(-1)                      # [B,M]
    gy = np.exp(-0.5 * y2)                      # [B,M]

    in_maps = []
    for c in range(8):
        b, nh = c // 2, c % 2
        nsl = slice(nh * NSH, (nh + 1) * NSH)
        xT = np.ascontiguousarray(x_[b, nsl].T, dtype=np.float32)        # [D, NSH]
        yT = np.ascontiguousarray(y_[b].T, dtype=np.float32)             # [D, M]
        xh, xl = _split(xT, xdt)
        yh = yT.astype(xdt)
        xhl = np.concatenate([xh, xl], axis=0)                            # [128, NSH]
        yhh = np.concatenate([yh, yh], axis=0)                            # [128, M]

        cx = np.cos(mx[b, nsl]).astype(np.float32)
        sx = np.sin(mx[b, nsl]).astype(np.float32)
        cy = (np.cos(my[b]) * gy[b]).astype(np.float32)
        sy = (np.sin(my[b]) * gy[b]).astype(np.float32)
        cxh, cxl = _split(cx, bf)
        sxh, sxl = _split(sx, bf)
        cyh, cyl = _split(cy, bf)
        syh, syl = _split(sy, bf)
        csx = np.stack([cxh, sxh, cxh, sxh, cxl, sxl])                    # [6, NSH]
        csy = np.stack([cyh, syh, cyl, syl, cyh, syh])                    # [6, M]

        bias = np.ascontiguousarray(
            (-0.5 * x2[b, nsl]).reshape(NB, 128).T, dtype=np.float32)    # [128, NB]
        im = dict(xhl=xhl, yhh=yhh, csx=csx, csy=csy, bias=bias)
        if scheme == "precise":
            yl = (yT - yh.astype(np.float32)).astype(bf)
            im["yl"] = yl
        in_maps.append(im)
    return in_maps


def kernel(x, y, mu, logs2diag):
    from concourse.bass_utils import run_bass_kernel_spmd

    nc = _build()
    in_maps = make_in_maps(x, y, mu, logs2diag)
    res = run_bass_kernel_spmd(nc, in_maps, core_ids=list(range(8)))

    out = np.empty((B, N, M), dtype=np.float32)
    for c in range(8):
        b, nh = c // 2, c % 2
        out[b, nh * NSH:(nh + 1) * NSH, :] = res.results[c]["out"]
    return out
